# revision 1
# baseline (speedup 1.0000x reference)
"""Trainium2 Bass kernel for nn_CausalPrefixAttention (8-core SPMD), v2.

Changes vs v1 (152us):
  - bf16 end-to-end: x/cx/weights DMA'd as bf16 (halves input DMA),
    activations (xT, q/k/v, p, vn, oT) held bf16 in SBUF, output DMA'd
    bf16 and upcast+summed on host. Matmuls accumulate f32 in PSUM;
    measured CPU-emulated rel err ~5e-3 vs the 2e-2 gate.
  - LayerNorm rs-scale moved off the x tiles: projections run on raw
    (bf16) x^T; the per-token rs is applied during the PSUM->SBUF copy of
    q/k/v via tensor_tensor with a PE-broadcast rs row (rank-1 matmul).
    Aug rows are now (-mu, std): q = rs*(x@W' - mu*u + std*b) == LN@W + b.
    This unblocks x transposes from the stats chain (DVE) entirely.
  - bf16 transposes run 1.0 cyc/row on PE (vs 1.5 f32r).
  - Final-projection PSUM->SBUF copies alternate DVE/ACT so the 2-bank
    ping-pong doesn't serialize PE on a single copy engine.
  - sim for the next g is emitted before the final projection of the
    current g so ACT keeps running exp during the out-proj block.
  - lrec/reciprocal/lbc merged: one [2,512] reciprocal, one sel-matmul.
  - transpose loops are tq-major so PE starts after the first 4 input
    tiles arrive rather than all 8.
"""

import os
import sys

for _p in ("/opt/trn_rl_repo", "/root/.axon_site/_ro/trn_rl_repo"):
    if os.path.isdir(_p) and _p not in sys.path:
        sys.path.append(_p)

import numpy as np

import concourse.mybir as mybir
import concourse.tile as tile
from concourse import bacc
from concourse.bass_utils import run_bass_kernel_spmd

F32 = mybir.dt.float32
F32R = mybir.dt.float32r
BF16 = mybir.dt.bfloat16
AF = mybir.ActivationFunctionType
ALU = mybir.AluOpType

B, N, M, DIM, INNER, HEADS, DH = 2, 1024, 1024, 1024, 512, 8, 64
EPS = 1e-5
NT = N // 128      # token tiles per batch (8)
KC = DIM // 128    # contraction chunks (8)


def build_program(unroll=1, phase=2):
    nc = bacc.Bacc("TRN2", target_bir_lowering=False, debug=False)

    x_d = nc.dram_tensor("x", [N, DIM], BF16, kind="ExternalInput")
    cx_d = nc.dram_tensor("cx", [M, DIM], BF16, kind="ExternalInput")
    # in-projection weights (gamma folded), chunks [128, 9, 384]:
    # chunk c rows = contraction rows 128c..128c+127; cols 0:128 q, 128:256 k,
    # 256:384 v. Chunk 8 rows 0/1 = the (u, b) rank-1 augmentation.
    win_d = nc.dram_tensor("win", [128, KC + 1, 384], BF16, kind="ExternalInput")
    # raw context projection weights, chunks [128, 8, 256]: 0:128 k, 128:256 v
    wcx_d = nc.dram_tensor("wcx", [128, KC, 256], BF16, kind="ExternalInput")
    wo_d = nc.dram_tensor("wo", [128, DIM], BF16, kind="ExternalInput")
    sel_d = nc.dram_tensor("sel", [2, 128], BF16, kind="ExternalInput")
    pick_d = nc.dram_tensor("pick", [3, 128], BF16, kind="ExternalInput")
    tri_d = nc.dram_tensor("tri", [128, 128], BF16, kind="ExternalInput")
    idf_d = nc.dram_tensor("idf", [128, 128], F32, kind="ExternalInput")
    idb_d = nc.dram_tensor("idb", [128, 128], BF16, kind="ExternalInput")
    o_d = nc.dram_tensor("o", [N, DIM], BF16, kind="ExternalOutput")

    with tile.TileContext(nc) as tc:
        for _ in range(unroll):
            _emit(nc, tc, x_d, cx_d, win_d, wcx_d, wo_d, sel_d, pick_d,
                  tri_d, idf_d, idb_d, o_d, phase)
    nc.compile()
    return nc


def _emit(nc, tc, x_d, cx_d, win_d, wcx_d, wo_d, sel_d, pick_d, tri_d,
          idf_d, idb_d, o_d, phase=2):
    from contextlib import ExitStack

    ctx = ExitStack()
    with ctx:
        consts = ctx.enter_context(tc.tile_pool(name="consts", bufs=1))
        wpool = ctx.enter_context(tc.tile_pool(name="wpool", bufs=1))
        projp = ctx.enter_context(tc.tile_pool(name="projp", bufs=5))
        vnp = ctx.enter_context(tc.tile_pool(name="vnp", bufs=4))
        ppool = ctx.enter_context(tc.tile_pool(name="ppool", bufs=3))
        otp = ctx.enter_context(tc.tile_pool(name="otp", bufs=2))
        ostp = ctx.enter_context(tc.tile_pool(name="ostp", bufs=2))
        tiny = ctx.enter_context(tc.tile_pool(name="tiny", bufs=8))

        ident = consts.tile([128, 128], F32)
        nc.gpsimd.dma_start(out=ident, in_=idf_d[:])
        identb = consts.tile([128, 128], BF16)
        nc.gpsimd.dma_start(out=identb, in_=idb_d[:])
        eps_col = consts.tile([128, 1], F32)
        nc.vector.memset(eps_col, EPS)
        ones_col2 = consts.tile([128, 8], BF16)
        nc.vector.memset(ones_col2, 1.0)
        # selector rows: sel[0] = [1]*64+[0]*64, sel[1] = [0]*64+[1]*64.
        # sel.T @ (1/l) broadcasts each head's 1/l row across its 64 rows.
        sel2 = consts.tile([1, 256], BF16)
        nc.gpsimd.dma_start(out=sel2, in_=sel_d[:])
        # lower-triangular-inclusive 0/1 mask (tri[j,i] = j<=i)
        tri = consts.tile([128, 128], BF16)
        nc.gpsimd.dma_start(out=tri, in_=tri_d[:])

        # weights
        win = wpool.tile([128, KC + 1, 384], BF16, tag="win")
        for hw_ in range(2):
            nc.gpsimd.dma_start(
                out=win[:, :, 192 * hw_:192 * hw_ + 192],
                in_=win_d[:, :, 192 * hw_:192 * hw_ + 192])
        wcx = wpool.tile([128, KC, 256], BF16, tag="wcx")
        nc.gpsimd.dma_start(out=wcx, in_=wcx_d[:])
        wo = wpool.tile([128, DIM], BF16, tag="wo")
        nc.gpsimd.dma_start(out=wo, in_=wo_d[:])

        # stat rows: row0 = -mu, row1 = std (aug contraction), row2 = rs
        srow = consts.tile([3, N], BF16)
        # pick3[2,:] = 1 selects srow row 2 (rs) in the broadcast matmul,
        # keeping every matmul/PSUM operand partition-0 aligned
        pick3 = consts.tile([3, 128], BF16)
        nc.gpsimd.dma_start(out=pick3, in_=pick_d[:])

        kcxT = projp.tile([128, M], BF16, tag="proj", name="kcxT")
        vcxT = projp.tile([128, M], BF16, tag="proj", name="vcxT")
        qT = projp.tile([128, N], BF16, tag="proj", name="qT")
        kinT = projp.tile([128, N], BF16, tag="proj", name="kinT")
        vinT = projp.tile([128, N], BF16, tag="proj", name="vinT")
        # rs broadcast [128, 512] per g-half, f32 in SBUF
        rsb = ctx.enter_context(tc.tile_pool(name="rsb", bufs=2))
        rs_bc = [rsb.tile([128, 512], F32, tag="rsbc", name=f"rsbc{g}")
                 for g in range(2)]
        vn = [None] * 16

        phase_a = ExitStack()
        with phase_a:
            natcx = phase_a.enter_context(tc.tile_pool(name="natcx", bufs=1))
            natx = phase_a.enter_context(tc.tile_pool(name="natx", bufs=1))
            tposed = phase_a.enter_context(tc.tile_pool(name="tposed", bufs=2))
            psA = phase_a.enter_context(
                tc.tile_pool(name="psA", bufs=1, space="PSUM"))

            def transpose_128(dst_t, srcs, ident_, copy_engines, ci0=0):
                # dst_t: [128, 2, KC, 512] bf16 tile with dims (token-half
                # tq, chunk c, token-col); srcs: 8 natural [128, 1024] bf16
                # tiles. PE-transpose 8 128x128 blocks (two chunks x 4
                # token-tiles) into one full PSUM bank, then a single
                # [128,1024] copy. tq-major so the first 4 src tiles
                # suffice to start.
                ci = ci0
                for tq in range(2):
                    for cp in range(0, KC, 2):
                        ps = psA.tile([128, 1024], BF16, tag="tps", bufs=3,
                                      name="tps")
                        for k in range(8):
                            c = cp + k // 4
                            t = tq * 4 + (k % 4)
                            nc.tensor.transpose(
                                ps[:, k * 128:(k + 1) * 128],
                                srcs[t][:, c * 128:(c + 1) * 128], ident_)
                        eng = copy_engines[ci % len(copy_engines)]
                        ci += 1
                        if eng == "act":
                            nc.scalar.copy(
                                out=dst_t[:, tq, cp:cp + 2, :], in_=ps)
                        else:
                            nc.vector.tensor_copy(
                                out=dst_t[:, tq, cp:cp + 2, :], in_=ps)

            # ---- context: load, transpose, cx projections ----
            cxnat_t = natcx.tile([128, NT, DIM], BF16, tag="nat", name="cxnat")
            cx_r = cx_d.rearrange("(t p) d -> p t d", p=128)
            for hf in range(NT):
                eng = nc.scalar if hf % 2 == 0 else nc.sync
                eng.dma_start(out=cxnat_t[:, hf:hf + 1, :],
                              in_=cx_r[:, hf:hf + 1, :])
            cx_nat = [cxnat_t[:, t, :] for t in range(NT)]
            # ---- x: load + stats (DVE/ACT); no in-place scale ----
            xnat_t = natx.tile([128, NT, DIM], BF16, tag="nat", name="xnat")
            x_r = x_d.rearrange("(t p) d -> p t d", p=128)
            for hf in range(NT):
                eng = nc.sync if hf % 2 == 0 else nc.scalar
                eng.dma_start(out=xnat_t[:, hf:hf + 1, :],
                              in_=x_r[:, hf:hf + 1, :])
            x_nat = [xnat_t[:, t, :] for t in range(NT)]
            stats4 = []
            for t in range(NT):
                xt = x_nat[t]
                s4 = tiny.tile([128, 4], F32, tag="s4", name=f"s4_{t}")
                stats4.append(s4)
                bst = tiny.tile([128, 2, 6], F32, tag="bst", name="bst")
                for half in range(2):
                    nc.vector.bn_stats(
                        out=bst[:, half, :],
                        in_=xt[:, half * 512:(half + 1) * 512])
                mv = tiny.tile([128, 2], F32, tag="mv", name="mv")
                nc.vector.bn_aggr(out=mv, in_=bst)
                # std = sqrt(var+eps) -> s4 col1
                nc.scalar.activation(
                    out=s4[:, 1:2], in_=mv[:, 1:2], func=AF.Sqrt, bias=eps_col)
                # rs = 1/std -> s4 col2
                nc.vector.reciprocal(out=s4[:, 2:3], in_=s4[:, 1:2])
                # -mu -> s4 col0
                nc.vector.tensor_scalar(
                    out=s4[:, 0:1], in0=mv[:, 0:1], scalar1=-1.0, scalar2=None,
                    op0=ALU.mult)

            cxT = tposed.tile([128, 2, KC, 512], BF16, tag="tp", name="cxT")
            transpose_128(cxT, cx_nat, identb, ("dve", "act", "dve"))

            for pj, dst in ((0, kcxT), (1, vcxT)):
                for g in range(2):
                    sp = slice(g * 512, (g + 1) * 512)
                    ps = psA.tile([128, 512], F32, tag="pps", bufs=3,
                                  name="pps")
                    for c in range(KC):
                        nc.tensor.matmul(
                            ps, wcx[:, c, pj * 128:(pj + 1) * 128],
                            cxT[:, g, c, :],
                            start=(c == 0), stop=(c == KC - 1))
                    if g == 0:
                        nc.vector.tensor_copy(out=dst[:, sp], in_=ps)
                    else:
                        nc.scalar.copy(out=dst[:, sp], in_=ps)

            # v_nat tiles: 4 j's per [128, 520] tile, each j = [64 vfeat h0 |
            # ones | 64 vfeat h1 | ones] so the PV stationary is contiguous.
            # One batched strided copy per 4 transposes.
            def v_transpose(src, base):
                for q in range(2):
                    v_t = vnp.tile([128, 520], BF16, tag="vn",
                                   name=f"vn{base + 4 * q}")
                    for jj in range(4):
                        vn[base + 4 * q + jj] = (v_t, jj)
                    ps = psA.tile([128, 512], BF16, tag="tpsr", bufs=2,
                                  name="tpsr")
                    for jj in range(4):
                        j = 4 * q + jj
                        nc.tensor.transpose(
                            ps[:, jj * 128:(jj + 1) * 128],
                            src[:, j * 128:(j + 1) * 128], identb)
                    nc.vector.tensor_copy(
                        out=v_t.rearrange("p (a b) -> p a b", b=65)[:, :, 64:65],
                        in_=ones_col2.rearrange("p (a b) -> p a b", b=1))
                    nc.vector.tensor_copy(
                        out=v_t.rearrange("p (a b) -> p a b", b=65)[:, :, 0:64],
                        in_=ps.rearrange("p (a b) -> p a b", b=64))

            v_transpose(vcxT, 0)

            # ---- x transposes (independent of stats now) ----
            xT = tposed.tile([128, 2, KC, 512], BF16, tag="tp", name="xT")
            transpose_128(xT, x_nat, identb, ("act", "dve", "dve"))

            # ---- stats rows: transpose s4 cols -> (-mu | std | rs) rows ----
            for t in range(NT):
                ps = psA.tile([128, 512], F32, tag="pps", bufs=3, name="pps")
                nc.tensor.transpose(ps[0:4, 0:128], stats4[t], ident)
                nc.vector.tensor_copy(
                    out=srow[:, t * 128:(t + 1) * 128], in_=ps[0:3, 0:128])
            # rs broadcast tiles via matmul: pick3^T selects srow row 2 (rs)
            # into every output partition
            for g in range(2):
                ps = psA.tile([128, 512], F32, tag="pps", bufs=3, name="pps")
                nc.tensor.matmul(
                    ps, pick3, srow[:, g * 512:(g + 1) * 512],
                    start=True, stop=True)
                nc.scalar.copy(out=rs_bc[g], in_=ps)

            # ---- input projections (q first so attention can start);
            # rs applied during the PSUM->SBUF copy ----
            for pj, dst in ((0, qT), (2, vinT), (1, kinT)):
                wsl = slice(pj * 128, (pj + 1) * 128)
                for g in range(2):
                    sp = slice(g * 512, (g + 1) * 512)
                    ps = psA.tile([128, 512], F32, tag="pps", bufs=3,
                                  name="pps")
                    for c in range(KC):
                        nc.tensor.matmul(
                            ps, win[:, c, wsl], xT[:, g, c, :],
                            start=(c == 0), stop=False)
                    nc.tensor.matmul(
                        ps, win[0:2, KC, wsl], srow[0:2, sp],
                        start=False, stop=True)
                    nc.vector.tensor_tensor(
                        out=dst[:, sp], in0=ps, in1=rs_bc[g], op=ALU.mult)

            v_transpose(vinT, 8)

            if phase == 1:
                for t, src_t in enumerate((qT, kinT, vinT, kcxT, vcxT,
                                           qT, kinT, vinT)):
                    nc.sync.dma_start(
                        out=o_d[t * 128:(t + 1) * 128, :].bitcast(BF16),
                        in_=src_t)
                return

        # ---- attention + final projection ----
        with tc.tile_pool(name="psSim", bufs=1, space="PSUM") as psS, \
             tc.tile_pool(name="psO", bufs=1, space="PSUM") as psO, \
             tc.tile_pool(name="psF", bufs=1, space="PSUM") as psF:
            pend_final = [None]

            def emit_final(g):
                # l rows -> 1/l -> broadcast to [128,512] via sel-matmul,
                # normalize o during PSUM->SBUF, then out-projection.
                o_ps = pend_final[0]
                lrec = [tiny.tile([1, 512], BF16, tag=f"lr{h}", bufs=2,
                                  name=f"lr{h}") for h in (0, 1)]
                with nc.allow_low_precision(reason="1/l in bf16 is plenty"):
                    for h in (0, 1):
                        nc.vector.tensor_copy(out=lrec[h],
                                              in_=o_ps[h][64:65, :])
                        nc.vector.reciprocal(out=lrec[h], in_=lrec[h])
                lbc_ps = psF.tile([128, 512], F32, tag="fin0", bufs=1,
                                  name="lbc")
                for h in (0, 1):
                    nc.tensor.matmul(lbc_ps, sel2[:, 128 * h:128 * h + 128],
                                     lrec[h], start=(h == 0), stop=(h == 1))
                lbc = tiny.tile([128, 512], F32, tag="lbc", bufs=2, name="lbc")
                nc.vector.tensor_copy(out=lbc, in_=lbc_ps)
                # normalized merged head outputs (bf16 for the final matmul)
                oT = otp.tile([128, 512], BF16, tag="oT")
                for h in (0, 1):
                    nc.vector.tensor_tensor(
                        out=oT[64 * h:64 * h + 64, :], in0=o_ps[h][0:64, :],
                        in1=lbc[64 * h:64 * h + 64, :], op=ALU.mult)

                o_r = o_d.rearrange("(t p) d -> p t d", p=128)
                for tp in range(2):
                    ost = ostp.tile([128, 2, DIM], BF16, tag="ost")
                    for ti in range(2):
                        t = tp * 2 + ti
                        for half in range(2):
                            wsp = slice(half * 512, (half + 1) * 512)
                            fp = psF.tile([128, 512], F32, tag=f"fin{half}",
                                          bufs=1, name=f"fin{half}")
                            nc.tensor.matmul(
                                fp, oT[:, t * 128:(t + 1) * 128], wo[:, wsp],
                                start=True, stop=True)
                            if half == 0:
                                nc.vector.tensor_copy(
                                    out=ost[:, ti, wsp], in_=fp)
                            else:
                                nc.scalar.copy(out=ost[:, ti, wsp], in_=fp)
                    eng = nc.sync if tp % 2 == 0 else nc.scalar
                    eng.dma_start(
                        out=o_r[:, g * 4 + tp * 2:g * 4 + tp * 2 + 2, :],
                        in_=ost)
                pend_final[0] = None

            for g in range(2):
                # j order: cx0..cx6, in0.., cx7 (start/stop on full spans)
                j_list = [("cx", j) for j in range(7)]
                j_list += [("in", j) for j in range(4 * g + 4)]
                j_list.append(("cx", 7))
                n_j = len(j_list)
                o_ps = [psO.tile([128, 512], F32, tag=f"o{h}", name=f"ops{h}")
                        for h in (0, 1)]

                def j_meta(idx, g=g, j_list=j_list):
                    src, j = j_list[idx]
                    if src == "cx":
                        return kcxT, j, j, 0, False
                    off = max(0, 128 * (j - 4 * g))
                    return kinT, j, 8 + j, off, j >= 4 * g

                sims = [None] * n_j

                def emit_sim(idx, j_meta=j_meta, sims=sims, g=g):
                    kT, j, jg, off, diag = j_meta(idx)
                    pair = []
                    for h in (0, 1):
                        hsl = slice(64 * h, 64 * h + 64)
                        ps = psS.tile([128, 512], F32, tag=f"sim{h}", bufs=2,
                                      name=f"sim{h}")
                        nc.tensor.matmul(
                            ps[:, off:512],
                            kT[hsl, j * 128:(j + 1) * 128],
                            qT[hsl, g * 512 + off:(g + 1) * 512],
                            start=True, stop=True)
                        pair.append(ps)
                    sims[idx] = pair

                # software pipeline: sim for j+1 is emitted before PV of j so
                # the in-order PE computes the next sim while ACT runs exp.
                emit_sim(0)
                # the previous g's out-projection goes here, AFTER the first
                # sim of this g, so ACT has exp work during the final block
                if pend_final[0] is not None:
                    emit_final(g - 1)
                for idx in range(n_j):
                    if idx + 1 < n_j:
                        emit_sim(idx + 1)
                    kT, j, jg, off, diag = j_meta(idx)
                    p_t = ppool.tile([128, 1024], BF16, tag="p", name="p")
                    for h in (0, 1):
                        nc.scalar.activation(
                            out=p_t[:, 512 * h + off:512 * (h + 1)],
                            in_=sims[idx][h][:, off:512], func=AF.Exp)
                    if diag:
                        for h in (0, 1):
                            nc.vector.tensor_tensor(
                                out=p_t[:, 512 * h + off:512 * h + off + 128],
                                in0=p_t[:, 512 * h + off:512 * h + off + 128],
                                in1=tri, op=ALU.mult)
                    sims[idx] = None
                    v_t, jj = vn[jg]
                    for h in (0, 1):
                        nc.tensor.matmul(
                            o_ps[h][0:65, off:512],
                            v_t[:, 130 * jj + 65 * h:130 * jj + 65 * h + 65],
                            p_t[:, 512 * h + off:512 * (h + 1)],
                            start=(idx == 0), stop=(idx == n_j - 1))
                pend_final[0] = o_ps
            emit_final(1)


_NC_CACHE = None


def _get_nc():
    global _NC_CACHE
    if _NC_CACHE is None:
        _NC_CACHE = build_program()
    return _NC_CACHE


def make_in_maps(x, context, gamma, beta, Wq, Wkv, Wo, bo):
    import ml_dtypes
    BF = ml_dtypes.bfloat16
    x = np.asarray(x, np.float32)
    context = np.asarray(context, np.float32)
    gamma = np.asarray(gamma, np.float32)
    beta = np.asarray(beta, np.float32)
    Wq = np.asarray(Wq, np.float32)
    Wkv = np.asarray(Wkv, np.float32)
    Wo = np.asarray(Wo, np.float32)

    s = DH ** -0.5
    in_maps = []
    for core in range(8):
        b, hg = divmod(core, 4)
        cols = slice(128 * hg, 128 * hg + 128)
        wq = Wq[:, cols] * gamma[:, None] * s
        uq = wq.sum(0)
        bq = beta @ Wq[:, cols] * s
        wk = Wkv[:, :INNER][:, cols] * gamma[:, None]
        uk = wk.sum(0)
        bk = beta @ Wkv[:, :INNER][:, cols]
        wv = Wkv[:, INNER:][:, cols] * gamma[:, None]
        uv = wv.sum(0)
        bv = beta @ Wkv[:, INNER:][:, cols]

        win = np.zeros((128, KC + 1, 384), np.float32)
        for c in range(KC):
            rows = slice(128 * c, 128 * c + 128)
            win[:, c, 0:128] = wq[rows]
            win[:, c, 128:256] = wk[rows]
            win[:, c, 256:384] = wv[rows]
        win[0, KC, 0:128] = uq
        win[1, KC, 0:128] = bq
        win[0, KC, 128:256] = uk
        win[1, KC, 128:256] = bk
        win[0, KC, 256:384] = uv
        win[1, KC, 256:384] = bv

        wcx = np.zeros((128, KC, 256), np.float32)
        for c in range(KC):
            rows = slice(128 * c, 128 * c + 128)
            wcx[:, c, 0:128] = Wkv[:, :INNER][rows, cols]
            wcx[:, c, 128:256] = Wkv[:, INNER:][rows, cols]

        sel = np.zeros((2, 128), np.float32)
        sel[0, 0:64] = 1.0
        sel[1, 64:128] = 1.0
        pick = np.zeros((3, 128), np.float32)
        pick[2, :] = 1.0
        tri = np.tril(np.ones((128, 128), np.float32)).T
        idm = np.eye(128, dtype=np.float32)
        in_maps.append({
            "idf": idm,
            "idb": idm.astype(BF),
            "pick": pick.astype(BF),
            "sel": sel.astype(BF),
            "tri": tri.astype(BF),
            "x": np.ascontiguousarray(x[b]).astype(BF),
            "cx": np.ascontiguousarray(context[b]).astype(BF),
            "win": win.astype(BF),
            "wcx": wcx.astype(BF),
            "wo": np.ascontiguousarray(Wo[cols, :]).astype(BF),
        })
    return in_maps


def assemble(results, bo):
    bo = np.asarray(bo, np.float32)
    out = np.zeros((B, N, DIM), np.float32)
    for core in range(8):
        b = core // 4
        out[b] += results[core]["o"].astype(np.float32)
    out += bo[None, None, :]
    return out


def kernel(x, context, gamma, beta, Wq, Wkv, Wo, bo):
    nc = _get_nc()
    in_maps = make_in_maps(x, context, gamma, beta, Wq, Wkv, Wo, bo)
    res = run_bass_kernel_spmd(nc, in_maps, list(range(8)))
    return assemble(res.results, bo)



# revision 6
# speedup vs baseline: 1.0014x; 1.0014x over previous
"""Trainium2 Bass kernel for nn_CausalPrefixAttention (8-core SPMD), v3.

Changes vs v2 (119.6us):
  - cx is never loaded in natural layout: 8 XBAR DMA-transposes load cxT
    straight from HBM into SBUF (16x128 tiles, 14ns/tile), removing 64 PE
    transposes and 8 big PSUM->SBUF copies.
  - x still loads natural (bn_stats needs tokens-on-partitions); PE
    transposes it during the otherwise DMA-bound head, with all 8
    PSUM->SBUF copies on ACT (idle then) and stats on DVE.
  - weights+consts packed into two blob DMAs (the old split win DMA paid
    the <512B-descriptor 2x latency penalty; blobs are 5-9KB/row).
  - sim PSUM is one [128,1024] f32 2-bank tile per j-tile (h0|h1), so exp
    is a single strided ACT instruction per j-tile instead of two (ACT
    per-instruction init is ~190ns; also halves the exp latency chain).
  - causal tri-masking moved from DVE to gpsimd (Pool), which is idle.
  - stats transpose runs in bf16 via the bf16 identity (f32 identity and
    its DMA dropped).
  - g order [1,0] so the bigger token-half's final overlaps g0's sims.
  - attention emission order (q, kin, vin, cx-proj, v-T, attention) keeps
    PE dense: the cost model halves PE clock for 3us after every stall.
"""

import os
import sys

for _p in ("/opt/trn_rl_repo", "/root/.axon_site/_ro/trn_rl_repo"):
    if os.path.isdir(_p) and _p not in sys.path:
        sys.path.append(_p)

import numpy as np

import concourse.mybir as mybir
import concourse.tile as tile
from concourse import bacc
from concourse.bass_utils import run_bass_kernel_spmd

F32 = mybir.dt.float32
BF16 = mybir.dt.bfloat16
AF = mybir.ActivationFunctionType
ALU = mybir.AluOpType

B, N, M, DIM, INNER, HEADS, DH = 2, 1024, 1024, 1024, 512, 8, 64
EPS = 1e-5
NT = N // 128      # token tiles per batch (8)
KC = DIM // 128    # contraction chunks (8)

# blob1 column offsets (bf16): wcx | idb | tri | pick | sel0 | sel1
B1_WCX, B1_IDB, B1_TRI, B1_PICK, B1_SEL = 0, 2048, 2176, 2304, 2432
B1_COLS = 2688
# blob2: win | wo
B2_WIN, B2_WO = 0, 3456
B2_COLS = 4480


def build_program(unroll=1, phase=2):
    nc = bacc.Bacc("TRN2", target_bir_lowering=False, debug=False)

    x_d = nc.dram_tensor("x", [N, DIM], BF16, kind="ExternalInput")
    cx_d = nc.dram_tensor("cx", [M, DIM], BF16, kind="ExternalInput")
    b1_d = nc.dram_tensor("b1", [128, B1_COLS], BF16, kind="ExternalInput")
    b2_d = nc.dram_tensor("b2", [128, B2_COLS], BF16, kind="ExternalInput")
    o_d = nc.dram_tensor("o", [N, DIM], BF16, kind="ExternalOutput")

    with tile.TileContext(nc) as tc:
        for _ in range(unroll):
            _emit(nc, tc, x_d, cx_d, b1_d, b2_d, o_d, phase)
    nc.compile()
    return nc


def _emit(nc, tc, x_d, cx_d, b1_d, b2_d, o_d, phase=2):
    from contextlib import ExitStack

    ctx = ExitStack()
    with ctx:
        wpool = ctx.enter_context(tc.tile_pool(name="wpool", bufs=1))
        projp = ctx.enter_context(tc.tile_pool(name="projp", bufs=5))
        vnp = ctx.enter_context(tc.tile_pool(name="vnp", bufs=4))
        ppool = ctx.enter_context(tc.tile_pool(name="ppool", bufs=3))
        otp = ctx.enter_context(tc.tile_pool(name="otp", bufs=2))
        ostp = ctx.enter_context(tc.tile_pool(name="ostp", bufs=2))
        tiny = ctx.enter_context(tc.tile_pool(name="tiny", bufs=8))
        consts = ctx.enter_context(tc.tile_pool(name="consts", bufs=1))

        eps_col = consts.tile([128, 1], F32)
        nc.vector.memset(eps_col, EPS)
        ones_col2 = consts.tile([128, 8], BF16)
        nc.vector.memset(ones_col2, 1.0)

        # ---- input DMA stream ----
        b1 = wpool.tile([128, B1_COLS], BF16, tag="b1")
        nc.sync.dma_start(out=b1, in_=b1_d[:])
        wcx = b1[:, B1_WCX:B1_WCX + 2048].rearrange("p (c k) -> p c k", k=256)
        identb = b1[:, B1_IDB:B1_IDB + 128]
        tri = b1[:, B1_TRI:B1_TRI + 128]
        pick3 = b1[0:3, B1_PICK:B1_PICK + 128]
        sel_h = [b1[0:1, B1_SEL + 128 * h:B1_SEL + 128 * h + 128] for h in (0, 1)]

        natx = ctx.enter_context(tc.tile_pool(name="natx", bufs=1))
        xnat_t = natx.tile([128, NT, DIM], BF16, tag="nat", name="xnat")
        x_r = x_d.rearrange("(t p) d -> p t d", p=128)
        for hf in range(NT):
            eng = nc.sync if hf % 2 == 0 else nc.scalar
            eng.dma_start(out=xnat_t[:, hf:hf + 1, :], in_=x_r[:, hf:hf + 1, :])
        x_nat = [xnat_t[:, t, :] for t in range(NT)]

        b2 = wpool.tile([128, B2_COLS], BF16, tag="b2")
        nc.scalar.dma_start(out=b2, in_=b2_d[:])
        win = b2[:, B2_WIN:B2_WIN + 3456].rearrange("p (c k) -> p c k", k=384)
        wo = b2[:, B2_WO:B2_WO + 1024]

        # cxT via XBAR DMA transpose, chunk-major so projections can stream.
        # ALL XBAR transposes must share one queue: two concurrent XBAR DMAs
        # on different queues corrupt each other (measured on device; per-16
        # token stripes of garbage). Regular DMAs on other queues are fine.
        cxT_t = wpool.tile([128, KC, M], BF16, tag="cxT")
        for c in range(KC):
            nc.sync.dma_start(out=cxT_t[:, c, :],
                              in_=cx_d[:, c * 128:(c + 1) * 128],
                              transpose=True)

        # stat rows: row0 = -mu, row1 = std (aug contraction), row2 = rs
        srow = consts.tile([3, N], BF16)

        kcxT = projp.tile([128, M], BF16, tag="proj", name="kcxT")
        vcxT = projp.tile([128, M], BF16, tag="proj", name="vcxT")
        qT = projp.tile([128, N], BF16, tag="proj", name="qT")
        kinT = projp.tile([128, N], BF16, tag="proj", name="kinT")
        vinT = projp.tile([128, N], BF16, tag="proj", name="vinT")
        rsb = ctx.enter_context(tc.tile_pool(name="rsb", bufs=2))
        rs_bc = [rsb.tile([128, 512], F32, tag="rsbc", name=f"rsbc{g}")
                 for g in range(2)]
        vn = [None] * 16

        phase_a = ExitStack()
        with phase_a:
            tposed = phase_a.enter_context(tc.tile_pool(name="tposed", bufs=1))
            psA = phase_a.enter_context(
                tc.tile_pool(name="psA", bufs=1, space="PSUM"))

            # ---- x transposes on PE (head filler); copies on ACT ----
            xT = tposed.tile([128, 2, KC, 512], BF16, tag="tp", name="xT")
            for tq in range(2):
                for cp in range(0, KC, 2):
                    ps = psA.tile([128, 1024], BF16, tag="tps", bufs=3,
                                  name="tps")
                    for k in range(8):
                        c = cp + k // 4
                        t = tq * 4 + (k % 4)
                        nc.tensor.transpose(
                            ps[:, k * 128:(k + 1) * 128],
                            x_nat[t][:, c * 128:(c + 1) * 128], identb)
                    nc.scalar.copy(out=xT[:, tq, cp:cp + 2, :], in_=ps)

            # ---- stats (DVE; after copies in DVE order, deps only on xnat) ----
            stats4 = []
            for t in range(NT):
                xt = x_nat[t]
                s4 = tiny.tile([128, 4], F32, tag="s4", name=f"s4_{t}")
                stats4.append(s4)
                bst = tiny.tile([128, 2, 6], F32, tag="bst", name="bst")
                for half in range(2):
                    nc.vector.bn_stats(
                        out=bst[:, half, :],
                        in_=xt[:, half * 512:(half + 1) * 512])
                mv = tiny.tile([128, 2], F32, tag="mv", name="mv")
                nc.vector.bn_aggr(out=mv, in_=bst)
                nc.scalar.activation(
                    out=s4[:, 1:2], in_=mv[:, 1:2], func=AF.Sqrt, bias=eps_col)
                nc.vector.reciprocal(out=s4[:, 2:3], in_=s4[:, 1:2])
                nc.vector.tensor_scalar(
                    out=s4[:, 0:1], in0=mv[:, 0:1], scalar1=-1.0, scalar2=None,
                    op0=ALU.mult)

            # ---- stats rows via bf16 transpose -> (-mu | std | rs) rows ----
            for t in range(NT):
                s4b = tiny.tile([128, 3], BF16, tag="s4b", name="s4b")
                nc.vector.tensor_copy(out=s4b, in_=stats4[t][:, 0:3])
                ps = psA.tile([128, 512], BF16, tag="tpsr", bufs=2, name="tpsr")
                nc.tensor.transpose(ps[0:3, 0:128], s4b, identb)
                nc.vector.tensor_copy(
                    out=srow[:, t * 128:(t + 1) * 128], in_=ps[0:3, 0:128])
            # rs broadcast tiles: pick3^T selects srow row 2 into every part
            for g in range(2):
                ps = psA.tile([128, 512], F32, tag="pps", bufs=3, name="pps")
                nc.tensor.matmul(
                    ps, pick3, srow[:, g * 512:(g + 1) * 512],
                    start=True, stop=True)
                nc.scalar.copy(out=rs_bc[g], in_=ps)

            # ---- input projections (q first); rs applied on PSUM->SBUF ----
            for pj, dst in ((0, qT), (1, kinT), (2, vinT)):
                wsl = slice(pj * 128, (pj + 1) * 128)
                for g in range(2):
                    sp = slice(g * 512, (g + 1) * 512)
                    ps = psA.tile([128, 512], F32, tag="pps", bufs=3,
                                  name="pps")
                    for c in range(KC):
                        nc.tensor.matmul(
                            ps, win[:, c, wsl], xT[:, g, c, :],
                            start=(c == 0), stop=False)
                    nc.tensor.matmul(
                        ps, win[0:2, KC, wsl], srow[0:2, sp],
                        start=False, stop=True)
                    nc.vector.tensor_tensor(
                        out=dst[:, sp], in0=ps, in1=rs_bc[g], op=ALU.mult)

            # ---- context projections (cxT streamed by DMA long before) ----
            for pj, dst in ((0, kcxT), (1, vcxT)):
                for g in range(2):
                    sp = slice(g * 512, (g + 1) * 512)
                    ps = psA.tile([128, 512], F32, tag="pps", bufs=3,
                                  name="pps")
                    for c in range(KC):
                        nc.tensor.matmul(
                            ps, wcx[:, c, pj * 128:(pj + 1) * 128],
                            cxT_t[:, c, sp],
                            start=(c == 0), stop=(c == KC - 1))
                    if g == 0:
                        nc.vector.tensor_copy(out=dst[:, sp], in_=ps)
                    else:
                        nc.scalar.copy(out=dst[:, sp], in_=ps)

            # v_nat tiles: 4 j's per [128, 520] tile, each j = [64 vfeat h0 |
            # ones | 64 vfeat h1 | ones] so the PV stationary is contiguous.
            def v_transpose(src, base):
                for q in range(2):
                    v_t = vnp.tile([128, 520], BF16, tag="vn",
                                   name=f"vn{base + 4 * q}")
                    for jj in range(4):
                        vn[base + 4 * q + jj] = (v_t, jj)
                    ps = psA.tile([128, 512], BF16, tag="tpsr", bufs=2,
                                  name="tpsr")
                    for jj in range(4):
                        j = 4 * q + jj
                        nc.tensor.transpose(
                            ps[:, jj * 128:(jj + 1) * 128],
                            src[:, j * 128:(j + 1) * 128], identb)
                    nc.vector.tensor_copy(
                        out=v_t.rearrange("p (a b) -> p a b", b=65)[:, :, 64:65],
                        in_=ones_col2.rearrange("p (a b) -> p a b", b=1))
                    nc.vector.tensor_copy(
                        out=v_t.rearrange("p (a b) -> p a b", b=65)[:, :, 0:64],
                        in_=ps.rearrange("p (a b) -> p a b", b=64))

            v_transpose(vinT, 8)
            v_transpose(vcxT, 0)

            if phase == 1:
                for t, src_t in enumerate((qT, kinT, vinT, kcxT, vcxT,
                                           qT, kinT, vinT)):
                    nc.sync.dma_start(
                        out=o_d[t * 128:(t + 1) * 128, :].bitcast(BF16),
                        in_=src_t)
                return
            if phase == 3:
                for t in range(NT):
                    nc.sync.dma_start(
                        out=o_d[t * 128:(t + 1) * 128, :].bitcast(BF16),
                        in_=cxT_t[:, t, :])
                return

        # ---- attention + final projection ----
        with tc.tile_pool(name="psSim", bufs=1, space="PSUM") as psS, \
             tc.tile_pool(name="psO", bufs=1, space="PSUM") as psO, \
             tc.tile_pool(name="psF", bufs=1, space="PSUM") as psF:
            pend_final = [None]

            def emit_final(g, o_ps):
                # l rows -> 1/l -> broadcast to [128,512] via sel-matmul,
                # normalize o during PSUM->SBUF, then out-projection.
                lrec = [tiny.tile([1, 512], BF16, tag=f"lr{h}", bufs=2,
                                  name=f"lr{h}") for h in (0, 1)]
                with nc.allow_low_precision(reason="1/l in bf16 is plenty"):
                    for h in (0, 1):
                        nc.vector.tensor_copy(out=lrec[h],
                                              in_=o_ps[h][64:65, :])
                        nc.vector.reciprocal(out=lrec[h], in_=lrec[h])
                lbc_ps = psF.tile([128, 512], F32, tag="fin0", bufs=1,
                                  name="lbc")
                for h in (0, 1):
                    nc.tensor.matmul(lbc_ps, sel_h[h], lrec[h],
                                     start=(h == 0), stop=(h == 1))
                lbc = tiny.tile([128, 512], F32, tag="lbc", bufs=2, name="lbc")
                nc.vector.tensor_copy(out=lbc, in_=lbc_ps)
                oT = otp.tile([128, 512], BF16, tag="oT")
                for h in (0, 1):
                    nc.vector.tensor_tensor(
                        out=oT[64 * h:64 * h + 64, :], in0=o_ps[h][0:64, :],
                        in1=lbc[64 * h:64 * h + 64, :], op=ALU.mult)

                o_r = o_d.rearrange("(t p) d -> p t d", p=128)
                for tp in range(2):
                    ost = ostp.tile([128, 2, DIM], BF16, tag="ost")
                    for ti in range(2):
                        t = tp * 2 + ti
                        for half in range(2):
                            wsp = slice(half * 512, (half + 1) * 512)
                            fp = psF.tile([128, 512], F32, tag=f"fin{half}",
                                          bufs=1, name=f"fin{half}")
                            nc.tensor.matmul(
                                fp, oT[:, t * 128:(t + 1) * 128], wo[:, wsp],
                                start=True, stop=True)
                            if half == 0:
                                nc.vector.tensor_copy(
                                    out=ost[:, ti, wsp], in_=fp)
                            else:
                                nc.scalar.copy(out=ost[:, ti, wsp], in_=fp)
                    eng = nc.sync if tp % 2 == 0 else nc.scalar
                    eng.dma_start(
                        out=o_r[:, g * 4 + tp * 2:g * 4 + tp * 2 + 2, :],
                        in_=ost)
                pend_final[0] = None

            for g in (1, 0):
                # j order: cx0..cx6, in0.., cx7 (start/stop on full spans)
                j_list = [("cx", j) for j in range(7)]
                j_list += [("in", j) for j in range(4 * g + 4)]
                j_list.append(("cx", 7))
                n_j = len(j_list)
                o_ps = [psO.tile([128, 512], F32, tag=f"o{h}", name=f"ops{h}")
                        for h in (0, 1)]

                def j_meta(idx, g=g, j_list=j_list):
                    src, j = j_list[idx]
                    if src == "cx":
                        return kcxT, j, j, 0, False
                    off = max(0, 128 * (j - 4 * g))
                    return kinT, j, 8 + j, off, j >= 4 * g

                sims = [None] * n_j

                def emit_sim(idx, j_meta=j_meta, sims=sims, g=g):
                    kT, j, jg, off, diag = j_meta(idx)
                    ps = psS.tile([128, 1024], F32, tag="sim", bufs=2,
                                  name="sim")
                    for h in (0, 1):
                        hsl = slice(64 * h, 64 * h + 64)
                        nc.tensor.matmul(
                            ps[:, 512 * h + off:512 * (h + 1)],
                            kT[hsl, j * 128:(j + 1) * 128],
                            qT[hsl, g * 512 + off:(g + 1) * 512],
                            start=True, stop=True)
                    sims[idx] = ps

                # software pipeline: sim for j+1 is emitted before PV of j so
                # the in-order PE computes the next sim while ACT runs exp.
                emit_sim(0)
                # the previous g's out-projection goes here, AFTER the first
                # sim of this g, so ACT has exp work during the final block
                if pend_final[0] is not None:
                    emit_final(1, pend_final[0])
                for idx in range(n_j):
                    if idx + 1 < n_j:
                        emit_sim(idx + 1)
                    kT, j, jg, off, diag = j_meta(idx)
                    p_t = ppool.tile([128, 1024], BF16, tag="p", name="p")
                    ps3 = sims[idx].rearrange("p (h t) -> p h t", h=2)
                    p3 = p_t.rearrange("p (h t) -> p h t", h=2)
                    nc.scalar.activation(
                        out=p3[:, :, off:512], in_=ps3[:, :, off:512],
                        func=AF.Exp)
                    if diag:
                        for h in (0, 1):
                            nc.gpsimd.tensor_tensor(
                                out=p_t[:, 512 * h + off:512 * h + off + 128],
                                in0=p_t[:, 512 * h + off:512 * h + off + 128],
                                in1=tri, op=ALU.mult)
                    sims[idx] = None
                    v_t, jj = vn[jg]
                    for h in (0, 1):
                        nc.tensor.matmul(
                            o_ps[h][0:65, off:512],
                            v_t[:, 130 * jj + 65 * h:130 * jj + 65 * h + 65],
                            p_t[:, 512 * h + off:512 * (h + 1)],
                            start=(idx == 0), stop=(idx == n_j - 1))
                pend_final[0] = o_ps
            emit_final(0, pend_final[0])


_NC_CACHE = None


def _get_nc():
    global _NC_CACHE
    if _NC_CACHE is None:
        _NC_CACHE = build_program()
    return _NC_CACHE


def make_in_maps(x, context, gamma, beta, Wq, Wkv, Wo, bo):
    import ml_dtypes
    BF = ml_dtypes.bfloat16
    x = np.asarray(x, np.float32)
    context = np.asarray(context, np.float32)
    gamma = np.asarray(gamma, np.float32)
    beta = np.asarray(beta, np.float32)
    Wq = np.asarray(Wq, np.float32)
    Wkv = np.asarray(Wkv, np.float32)
    Wo = np.asarray(Wo, np.float32)

    s = DH ** -0.5
    in_maps = []
    for core in range(8):
        b, hg = divmod(core, 4)
        cols = slice(128 * hg, 128 * hg + 128)
        wq = Wq[:, cols] * gamma[:, None] * s
        uq = wq.sum(0)
        bq = beta @ Wq[:, cols] * s
        wk = Wkv[:, :INNER][:, cols] * gamma[:, None]
        uk = wk.sum(0)
        bk = beta @ Wkv[:, :INNER][:, cols]
        wv = Wkv[:, INNER:][:, cols] * gamma[:, None]
        uv = wv.sum(0)
        bv = beta @ Wkv[:, INNER:][:, cols]

        win = np.zeros((128, KC + 1, 384), np.float32)
        for c in range(KC):
            rows = slice(128 * c, 128 * c + 128)
            win[:, c, 0:128] = wq[rows]
            win[:, c, 128:256] = wk[rows]
            win[:, c, 256:384] = wv[rows]
        win[0, KC, 0:128] = uq
        win[1, KC, 0:128] = bq
        win[0, KC, 128:256] = uk
        win[1, KC, 128:256] = bk
        win[0, KC, 256:384] = uv
        win[1, KC, 256:384] = bv

        wcx = np.zeros((128, KC, 256), np.float32)
        for c in range(KC):
            rows = slice(128 * c, 128 * c + 128)
            wcx[:, c, 0:128] = Wkv[:, :INNER][rows, cols]
            wcx[:, c, 128:256] = Wkv[:, INNER:][rows, cols]

        b1 = np.zeros((128, B1_COLS), np.float32)
        b1[:, B1_WCX:B1_WCX + 2048] = wcx.reshape(128, 2048)
        b1[:, B1_IDB:B1_IDB + 128] = np.eye(128, dtype=np.float32)
        b1[:, B1_TRI:B1_TRI + 128] = np.tril(np.ones((128, 128), np.float32)).T
        b1[2, B1_PICK:B1_PICK + 128] = 1.0
        b1[0, B1_SEL:B1_SEL + 64] = 1.0
        b1[0, B1_SEL + 192:B1_SEL + 256] = 1.0

        b2 = np.zeros((128, B2_COLS), np.float32)
        b2[:, B2_WIN:B2_WIN + 3456] = win.reshape(128, 3456)
        b2[:, B2_WO:B2_WO + 1024] = Wo[cols, :]

        in_maps.append({
            "x": np.ascontiguousarray(x[b]).astype(BF),
            "cx": np.ascontiguousarray(context[b]).astype(BF),
            "b1": b1.astype(BF),
            "b2": b2.astype(BF),
        })
    return in_maps


def assemble(results, bo):
    bo = np.asarray(bo, np.float32)
    out = np.zeros((B, N, DIM), np.float32)
    for core in range(8):
        b = core // 4
        out[b] += results[core]["o"].astype(np.float32)
    out += bo[None, None, :]
    return out


def kernel(x, context, gamma, beta, Wq, Wkv, Wo, bo):
    nc = _get_nc()
    in_maps = make_in_maps(x, context, gamma, beta, Wq, Wkv, Wo, bo)
    res = run_bass_kernel_spmd(nc, in_maps, list(range(8)))
    return assemble(res.results, bo)


# revision 10
# speedup vs baseline: 1.0459x; 1.0445x over previous
"""Trainium2 Bass kernel for nn_CausalPrefixAttention (8-core SPMD), v3.1.

Changes vs v2 (119.6us):
  - cx is never loaded in natural layout: 8 XBAR DMA-transposes load cxT
    straight from HBM into SBUF, removing 64 PE transposes and 8 big
    PSUM->SBUF copies. ALL XBAR transposes share one queue: two concurrent
    XBAR DMAs on different queues corrupt each other (measured on device;
    per-16-token stripes of garbage). Regular DMAs on other queues are ok.
  - x still loads natural (bn_stats needs tokens-on-partitions); PE
    transposes it per-tile during the otherwise DMA-bound head (first PSUM
    batch needs only x tile 0), with all 8 PSUM->SBUF copies on ACT
    (idle then) and stats on DVE.
  - weights+consts packed into blob DMAs; win's q-block is a separate DMA
    so the q projection can start before the k/v blocks land.
  - sim PSUM is one [128,1024] f32 2-bank tile per j-tile (h0|h1), so exp
    is a single strided ACT instruction per j-tile instead of two.
  - causal tri-masking on gpsimd (Pool); out-projection PSUM->SBUF copies
    on DVE, keeping ACT = pure exp during attention.
  - final: both heads' 1/l in one reciprocal + one sel-matmul.
  - emission order (x-T, stats, cx-proj, q/k/v-proj, attention) matches
    DMA arrival so the in-order PE rarely stalls: the cost model halves PE
    clock for 3us after every stall.
"""

import os
import sys

for _p in ("/opt/trn_rl_repo", "/root/.axon_site/_ro/trn_rl_repo"):
    if os.path.isdir(_p) and _p not in sys.path:
        sys.path.append(_p)

import numpy as np

import concourse.mybir as mybir
import concourse.tile as tile
from concourse import bacc
from concourse.bass_utils import run_bass_kernel_spmd

F32 = mybir.dt.float32
BF16 = mybir.dt.bfloat16
AF = mybir.ActivationFunctionType
ALU = mybir.AluOpType

B, N, M, DIM, INNER, HEADS, DH = 2, 1024, 1024, 1024, 512, 8, 64
EPS = 1e-5
NT = N // 128      # token tiles per batch (8)
KC = DIM // 128    # contraction chunks (8)

# blob1 column offsets (bf16): wcx | idb | tri | pick | sel (row 0, 2x128)
B1_WCX, B1_IDB, B1_TRI, B1_PICK, B1_SEL = 0, 2048, 2176, 2304, 2432
B1_COLS = 2688
# blob2: win q-block | k-block | v-block | wo (split DMA: q early, rest later)
B2_WQ, B2_WK, B2_WV, B2_WO = 0, 1152, 2304, 3456
B2_COLS = 4480


def build_program(unroll=1, phase=2):
    nc = bacc.Bacc("TRN2", target_bir_lowering=False, debug=False)

    x_d = nc.dram_tensor("x", [N, DIM], BF16, kind="ExternalInput")
    cx_d = nc.dram_tensor("cx", [M, DIM], BF16, kind="ExternalInput")
    b1_d = nc.dram_tensor("b1", [128, B1_COLS], BF16, kind="ExternalInput")
    b2_d = nc.dram_tensor("b2", [128, B2_COLS], BF16, kind="ExternalInput")
    o_d = nc.dram_tensor("o", [N, DIM], BF16, kind="ExternalOutput")

    with tile.TileContext(nc) as tc:
        for _ in range(unroll):
            _emit(nc, tc, x_d, cx_d, b1_d, b2_d, o_d, phase)
    nc.compile()
    return nc


def _emit(nc, tc, x_d, cx_d, b1_d, b2_d, o_d, phase=2):
    from contextlib import ExitStack

    ctx = ExitStack()
    with ctx:
        wpool = ctx.enter_context(tc.tile_pool(name="wpool", bufs=1))
        projp = ctx.enter_context(tc.tile_pool(name="projp", bufs=5))
        vnp = ctx.enter_context(tc.tile_pool(name="vnp", bufs=4))
        ppool = ctx.enter_context(tc.tile_pool(name="ppool", bufs=3))
        otp = ctx.enter_context(tc.tile_pool(name="otp", bufs=2))
        ostp = ctx.enter_context(tc.tile_pool(name="ostp", bufs=2))
        tiny = ctx.enter_context(tc.tile_pool(name="tiny", bufs=8))
        consts = ctx.enter_context(tc.tile_pool(name="consts", bufs=1))

        eps_col = consts.tile([128, 1], F32)
        nc.vector.memset(eps_col, EPS)
        ones_col2 = consts.tile([128, 8], BF16)
        nc.vector.memset(ones_col2, 1.0)

        # ---- input DMA stream: b1, x-nat tiles, cxT transposes, b2 ----
        b1 = wpool.tile([128, B1_COLS], BF16, tag="b1")
        nc.sync.dma_start(out=b1, in_=b1_d[:])
        wcx = b1[:, B1_WCX:B1_WCX + 2048].rearrange("p (c k) -> p c k", k=256)
        identb = b1[:, B1_IDB:B1_IDB + 128]
        tri = b1[:, B1_TRI:B1_TRI + 128]
        pick3 = b1[0:3, B1_PICK:B1_PICK + 128]
        sel2 = b1[0:1, B1_SEL:B1_SEL + 256]

        natx = ctx.enter_context(tc.tile_pool(name="natx", bufs=1))
        xnat_t = natx.tile([128, NT, DIM], BF16, tag="nat", name="xnat")
        x_r = x_d.rearrange("(t p) d -> p t d", p=128)
        for hf in range(NT):
            eng = nc.sync if hf % 2 == 0 else nc.scalar
            eng.dma_start(out=xnat_t[:, hf:hf + 1, :], in_=x_r[:, hf:hf + 1, :])
        x_nat = [xnat_t[:, t, :] for t in range(NT)]

        # cxT via XBAR DMA transpose, chunk-major (single queue — see above)
        cxT_t = wpool.tile([128, KC, M], BF16, tag="cxT")
        for c in range(KC):
            nc.sync.dma_start(out=cxT_t[:, c, :],
                              in_=cx_d[:, c * 128:(c + 1) * 128],
                              transpose=True)

        b2 = wpool.tile([128, B2_COLS], BF16, tag="b2")
        nc.scalar.dma_start(out=b2[:, 0:B2_WK], in_=b2_d[:, 0:B2_WK])
        nc.scalar.dma_start(out=b2[:, B2_WK:], in_=b2_d[:, B2_WK:])
        winq = b2[:, B2_WQ:B2_WQ + 1152].rearrange("p (c k) -> p c k", k=128)
        wink = b2[:, B2_WK:B2_WK + 1152].rearrange("p (c k) -> p c k", k=128)
        winv = b2[:, B2_WV:B2_WV + 1152].rearrange("p (c k) -> p c k", k=128)
        wo = b2[:, B2_WO:B2_WO + 1024]

        # stat rows: row0 = -mu, row1 = std (aug contraction), row2 = rs
        srow = consts.tile([3, N], BF16)

        kcxT = projp.tile([128, M], BF16, tag="proj", name="kcxT")
        vcxT = projp.tile([128, M], BF16, tag="proj", name="vcxT")
        qT = projp.tile([128, N], BF16, tag="proj", name="qT")
        kinT = projp.tile([128, N], BF16, tag="proj", name="kinT")
        vinT = projp.tile([128, N], BF16, tag="proj", name="vinT")
        rsb = ctx.enter_context(tc.tile_pool(name="rsb", bufs=2))
        rs_bc = [rsb.tile([128, 512], F32, tag="rsbc", name=f"rsbc{g}")
                 for g in range(2)]
        vn = [None] * 16

        phase_a = ExitStack()
        with phase_a:
            tposed = phase_a.enter_context(tc.tile_pool(name="tposed", bufs=1))
            psA = phase_a.enter_context(
                tc.tile_pool(name="psA", bufs=1, space="PSUM"))

            # ---- x transposes on PE, one x-tile per PSUM batch so the
            # first batch only needs x tile 0; copies on ACT ----
            xT = tposed.tile([128, 2, KC, 512], BF16, tag="tp", name="xT")
            for t in range(NT):
                ps = psA.tile([128, 1024], BF16, tag="tps", bufs=3,
                              name="tps")
                for c in range(KC):
                    nc.tensor.transpose(
                        ps[:, c * 128:(c + 1) * 128],
                        x_nat[t][:, c * 128:(c + 1) * 128], identb)
                co = (t % 4) * 128
                nc.scalar.copy(
                    out=xT[:, t // 4, :, co:co + 128],
                    in_=ps.rearrange("p (c k) -> p c k", k=128))

            # ---- stats (DVE) ----
            stats4 = []
            for t in range(NT):
                xt = x_nat[t]
                s4 = tiny.tile([128, 4], F32, tag="s4", name=f"s4_{t}")
                stats4.append(s4)
                bst = tiny.tile([128, 2, 6], F32, tag="bst", name="bst")
                for half in range(2):
                    nc.vector.bn_stats(
                        out=bst[:, half, :],
                        in_=xt[:, half * 512:(half + 1) * 512])
                mv = tiny.tile([128, 2], F32, tag="mv", name="mv")
                nc.vector.bn_aggr(out=mv, in_=bst)
                nc.scalar.activation(
                    out=s4[:, 1:2], in_=mv[:, 1:2], func=AF.Sqrt, bias=eps_col)
                nc.vector.reciprocal(out=s4[:, 2:3], in_=s4[:, 1:2])
                nc.vector.tensor_scalar(
                    out=s4[:, 0:1], in0=mv[:, 0:1], scalar1=-1.0, scalar2=None,
                    op0=ALU.mult)

            # ---- stats rows via bf16 transpose -> (-mu | std | rs) rows ----
            for t in range(NT):
                s4b = tiny.tile([128, 3], BF16, tag="s4b", name="s4b")
                nc.vector.tensor_copy(out=s4b, in_=stats4[t][:, 0:3])
                ps = psA.tile([128, 512], BF16, tag="tpsr", bufs=2, name="tpsr")
                nc.tensor.transpose(ps[0:3, 0:128], s4b, identb)
                nc.vector.tensor_copy(
                    out=srow[:, t * 128:(t + 1) * 128], in_=ps[0:3, 0:128])
            # rs broadcast tiles: pick3^T selects srow row 2 into every part
            for g in range(2):
                ps = psA.tile([128, 512], F32, tag="pps", bufs=3, name="pps")
                nc.tensor.matmul(
                    ps, pick3, srow[:, g * 512:(g + 1) * 512],
                    start=True, stop=True)
                nc.scalar.copy(out=rs_bc[g], in_=ps)

            # v_nat tiles: 4 j's per [128, 520] tile, each j = [64 vfeat h0 |
            # ones | 64 vfeat h1 | ones] so the PV stationary is contiguous.
            def v_transpose(src, base):
                for q in range(2):
                    v_t = vnp.tile([128, 520], BF16, tag="vn",
                                   name=f"vn{base + 4 * q}")
                    for jj in range(4):
                        vn[base + 4 * q + jj] = (v_t, jj)
                    ps = psA.tile([128, 512], BF16, tag="tpsr", bufs=2,
                                  name="tpsr")
                    for jj in range(4):
                        j = 4 * q + jj
                        nc.tensor.transpose(
                            ps[:, jj * 128:(jj + 1) * 128],
                            src[:, j * 128:(j + 1) * 128], identb)
                    nc.vector.tensor_copy(
                        out=v_t.rearrange("p (a b) -> p a b", b=65)[:, :, 64:65],
                        in_=ones_col2.rearrange("p (a b) -> p a b", b=1))
                    nc.vector.tensor_copy(
                        out=v_t.rearrange("p (a b) -> p a b", b=65)[:, :, 0:64],
                        in_=ps.rearrange("p (a b) -> p a b", b=64))

            # ---- context projections (cxT streamed by the DMA queue) ----
            for pj, dst in ((0, kcxT), (1, vcxT)):
                for g in range(2):
                    sp = slice(g * 512, (g + 1) * 512)
                    ps = psA.tile([128, 512], F32, tag="pps", bufs=3,
                                  name="pps")
                    for c in range(KC):
                        nc.tensor.matmul(
                            ps, wcx[:, c, pj * 128:(pj + 1) * 128],
                            cxT_t[:, c, sp],
                            start=(c == 0), stop=(c == KC - 1))
                    if g == 0:
                        nc.vector.tensor_copy(out=dst[:, sp], in_=ps)
                    else:
                        nc.scalar.copy(out=dst[:, sp], in_=ps)

            v_transpose(vcxT, 0)

            # ---- input projections (q first); rs applied on PSUM->SBUF ----
            for w9, dst in ((winq, qT), (wink, kinT), (winv, vinT)):
                for g in range(2):
                    sp = slice(g * 512, (g + 1) * 512)
                    ps = psA.tile([128, 512], F32, tag="pps", bufs=3,
                                  name="pps")
                    for c in range(KC):
                        nc.tensor.matmul(
                            ps, w9[:, c, :], xT[:, g, c, :],
                            start=(c == 0), stop=False)
                    nc.tensor.matmul(
                        ps, w9[0:2, KC, :], srow[0:2, sp],
                        start=False, stop=True)
                    nc.vector.tensor_tensor(
                        out=dst[:, sp], in0=ps, in1=rs_bc[g], op=ALU.mult)

            v_transpose(vinT, 8)

            if phase == 1:
                for t, src_t in enumerate((qT, kinT, vinT, kcxT, vcxT,
                                           qT, kinT, vinT)):
                    nc.sync.dma_start(
                        out=o_d[t * 128:(t + 1) * 128, :].bitcast(BF16),
                        in_=src_t)
                return

        # ---- attention + final projection ----
        with tc.tile_pool(name="psSim", bufs=1, space="PSUM") as psS, \
             tc.tile_pool(name="psO", bufs=1, space="PSUM") as psO, \
             tc.tile_pool(name="psF", bufs=1, space="PSUM") as psF:
            pend_final = [None]

            def emit_final(g, o_ps):
                # l rows -> 1/l -> broadcast to [128,512] via sel-matmuls,
                # normalize o during PSUM->SBUF, then out-projection.
                lrec = [tiny.tile([1, 512], BF16, tag=f"lr{h}", bufs=2,
                                  name=f"lr{h}") for h in (0, 1)]
                with nc.allow_low_precision(reason="1/l in bf16 is plenty"):
                    for h in (0, 1):
                        nc.vector.tensor_copy(out=lrec[h],
                                              in_=o_ps[h][64:65, :])
                        nc.vector.reciprocal(out=lrec[h], in_=lrec[h])
                lbc_ps = psF.tile([128, 512], F32, tag="fin0", bufs=1,
                                  name="lbc")
                for h in (0, 1):
                    nc.tensor.matmul(
                        lbc_ps, sel2[:, 128 * h:128 * h + 128], lrec[h],
                        start=(h == 0), stop=(h == 1))
                lbc = tiny.tile([128, 512], F32, tag="lbc", bufs=2, name="lbc")
                nc.vector.tensor_copy(out=lbc, in_=lbc_ps)
                oT = otp.tile([128, 512], BF16, tag="oT")
                for h in (0, 1):
                    nc.vector.tensor_tensor(
                        out=oT[64 * h:64 * h + 64, :], in0=o_ps[h][0:64, :],
                        in1=lbc[64 * h:64 * h + 64, :], op=ALU.mult)

                o_r = o_d.rearrange("(t p) d -> p t d", p=128)
                for tp in range(2):
                    ost = ostp.tile([128, 2, DIM], BF16, tag="ost")
                    for ti in range(2):
                        t = tp * 2 + ti
                        for half in range(2):
                            wsp = slice(half * 512, (half + 1) * 512)
                            fp = psF.tile([128, 512], F32, tag=f"fin{half}",
                                          bufs=1, name=f"fin{half}")
                            nc.tensor.matmul(
                                fp, oT[:, t * 128:(t + 1) * 128], wo[:, wsp],
                                start=True, stop=True)
                            nc.vector.tensor_copy(out=ost[:, ti, wsp], in_=fp)
                    eng = nc.sync if tp % 2 == 0 else nc.scalar
                    eng.dma_start(
                        out=o_r[:, g * 4 + tp * 2:g * 4 + tp * 2 + 2, :],
                        in_=ost)
                pend_final[0] = None

            prev_g = [None]
            for g in (0, 1):
                # j order: cx0..cx6, in0.., cx7 (start/stop on full spans)
                j_list = [("cx", j) for j in range(7)]
                j_list += [("in", j) for j in range(4 * g + 4)]
                j_list.append(("cx", 7))
                n_j = len(j_list)
                o_ps = [psO.tile([128, 512], F32, tag=f"o{h}", name=f"ops{h}")
                        for h in (0, 1)]

                def j_meta(idx, g=g, j_list=j_list):
                    src, j = j_list[idx]
                    if src == "cx":
                        return kcxT, j, j, 0, False
                    off = max(0, 128 * (j - 4 * g))
                    return kinT, j, 8 + j, off, j >= 4 * g

                sims = [None] * n_j

                def emit_sim(idx, j_meta=j_meta, sims=sims, g=g):
                    kT, j, jg, off, diag = j_meta(idx)
                    ps = psS.tile([128, 1024], F32, tag="sim", bufs=2,
                                  name="sim")
                    for h in (0, 1):
                        hsl = slice(64 * h, 64 * h + 64)
                        nc.tensor.matmul(
                            ps[:, 512 * h + off:512 * (h + 1)],
                            kT[hsl, j * 128:(j + 1) * 128],
                            qT[hsl, g * 512 + off:(g + 1) * 512],
                            start=True, stop=True)
                    sims[idx] = ps

                # software pipeline: sim for j+1 is emitted before PV of j so
                # the in-order PE computes the next sim while ACT runs exp.
                emit_sim(0)
                # the previous g's out-projection goes here, AFTER the first
                # sim of this g, so ACT has exp work during the final block
                if pend_final[0] is not None:
                    emit_final(prev_g[0], pend_final[0])
                for idx in range(n_j):
                    if idx + 1 < n_j:
                        emit_sim(idx + 1)
                    kT, j, jg, off, diag = j_meta(idx)
                    p_t = ppool.tile([128, 1024], BF16, tag="p", name="p")
                    ps3 = sims[idx].rearrange("p (h t) -> p h t", h=2)
                    p3 = p_t.rearrange("p (h t) -> p h t", h=2)
                    nc.scalar.activation(
                        out=p3[:, :, off:512], in_=ps3[:, :, off:512],
                        func=AF.Exp)
                    if diag:
                        for h in (0, 1):
                            nc.gpsimd.tensor_tensor(
                                out=p_t[:, 512 * h + off:512 * h + off + 128],
                                in0=p_t[:, 512 * h + off:512 * h + off + 128],
                                in1=tri, op=ALU.mult)
                    sims[idx] = None
                    v_t, jj = vn[jg]
                    for h in (0, 1):
                        nc.tensor.matmul(
                            o_ps[h][0:65, off:512],
                            v_t[:, 130 * jj + 65 * h:130 * jj + 65 * h + 65],
                            p_t[:, 512 * h + off:512 * (h + 1)],
                            start=(idx == 0), stop=(idx == n_j - 1))
                pend_final[0] = o_ps
                prev_g[0] = g
            emit_final(1, pend_final[0])


_NC_CACHE = None


def _get_nc():
    global _NC_CACHE
    if _NC_CACHE is None:
        _NC_CACHE = build_program()
    return _NC_CACHE


def make_in_maps(x, context, gamma, beta, Wq, Wkv, Wo, bo):
    import ml_dtypes
    BF = ml_dtypes.bfloat16
    x = np.asarray(x, np.float32)
    context = np.asarray(context, np.float32)
    gamma = np.asarray(gamma, np.float32)
    beta = np.asarray(beta, np.float32)
    Wq = np.asarray(Wq, np.float32)
    Wkv = np.asarray(Wkv, np.float32)
    Wo = np.asarray(Wo, np.float32)

    s = DH ** -0.5
    in_maps = []
    for core in range(8):
        b, hg = divmod(core, 4)
        cols = slice(128 * hg, 128 * hg + 128)
        wq = Wq[:, cols] * gamma[:, None] * s
        uq = wq.sum(0)
        bq = beta @ Wq[:, cols] * s
        wk = Wkv[:, :INNER][:, cols] * gamma[:, None]
        uk = wk.sum(0)
        bk = beta @ Wkv[:, :INNER][:, cols]
        wv = Wkv[:, INNER:][:, cols] * gamma[:, None]
        uv = wv.sum(0)
        bv = beta @ Wkv[:, INNER:][:, cols]

        # per-projection 9-chunk blocks (chunk 8 = aug rows u, b)
        def blk(w, u, bvec):
            out = np.zeros((128, KC + 1, 128), np.float32)
            for c in range(KC):
                out[:, c, :] = w[128 * c:128 * c + 128]
            out[0, KC, :] = u
            out[1, KC, :] = bvec
            return out.reshape(128, 1152)

        wcx = np.zeros((128, KC, 256), np.float32)
        for c in range(KC):
            rows = slice(128 * c, 128 * c + 128)
            wcx[:, c, 0:128] = Wkv[:, :INNER][rows, cols]
            wcx[:, c, 128:256] = Wkv[:, INNER:][rows, cols]

        b1 = np.zeros((128, B1_COLS), np.float32)
        b1[:, B1_WCX:B1_WCX + 2048] = wcx.reshape(128, 2048)
        b1[:, B1_IDB:B1_IDB + 128] = np.eye(128, dtype=np.float32)
        b1[:, B1_TRI:B1_TRI + 128] = np.tril(np.ones((128, 128), np.float32)).T
        b1[2, B1_PICK:B1_PICK + 128] = 1.0
        b1[0, B1_SEL:B1_SEL + 64] = 1.0
        b1[0, B1_SEL + 192:B1_SEL + 256] = 1.0

        b2 = np.zeros((128, B2_COLS), np.float32)
        b2[:, B2_WQ:B2_WQ + 1152] = blk(wq, uq, bq)
        b2[:, B2_WK:B2_WK + 1152] = blk(wk, uk, bk)
        b2[:, B2_WV:B2_WV + 1152] = blk(wv, uv, bv)
        b2[:, B2_WO:B2_WO + 1024] = Wo[cols, :]

        in_maps.append({
            "x": np.ascontiguousarray(x[b]).astype(BF),
            "cx": np.ascontiguousarray(context[b]).astype(BF),
            "b1": b1.astype(BF),
            "b2": b2.astype(BF),
        })
    return in_maps


def assemble(results, bo):
    bo = np.asarray(bo, np.float32)
    out = np.zeros((B, N, DIM), np.float32)
    for core in range(8):
        b = core // 4
        out[b] += results[core]["o"].astype(np.float32)
    out += bo[None, None, :]
    return out


def kernel(x, context, gamma, beta, Wq, Wkv, Wo, bo):
    nc = _get_nc()
    in_maps = make_in_maps(x, context, gamma, beta, Wq, Wkv, Wo, bo)
    res = run_bass_kernel_spmd(nc, in_maps, list(range(8)))
    return assemble(res.results, bo)
